# revision 36
# baseline (speedup 1.0000x reference)
"""Self-contained Trainium2 Bass kernel for nn_Classifier_79929341379065.

kernel(**inputs) takes FULL unsharded inputs (as produced by
reference.setup_inputs()) and returns the FULL [B, 1] float32 output.
Internally: pure data parallel over 8 NeuronCores (batch dim of x),
weights replicated.

Hardcoded shapes: B=8192, L=16, H=8, DK=DV=32, DM=256, BN=128, V=50000.
Per core: 1024 batches = 16384 tokens = 128 subtiles of 128 tokens
(each subtile = 8 attention groups of L=16), processed in blocks of
4 subtiles (512 tokens).

Per subtile (token-major attention, as the tokens arrive from the
gather): one indirect gather from a merged table [V, 256]
(LN-normalized | raw-with-row0-zeroed); PE transposes to feature-major;
k / head-masked q4 projections; S^T (4 heads per matmul, K=128);
exp * blockdiag-minus-eye mask; PV with ones-augmented v giving
[ctx~|den]; normalize; PE-transpose ctx into a block-wide ctxT.

Per block (feature-major, 512-token-wide weight-stationary matmuls):
fc1 -> tanh MLP -> +residual, static tanh MLP from emT, then the
classifier head in closed form: with a=rstd_u, b=rstd_s,
c=a*mu_u-b*mu_s per token,
  logit = a^2*Swuu + b^2*Swss + c^2*Sw - 2*(ab*Swust + c*(a*Swu-b*Swst))
where all S* are partition-contractions computed by 10 matmuls
accumulating into one [10, 512] PSUM bank with [wcls|ones] column
blocks; a 10x128 PE transpose brings the stats token-major for the
final scalar math, sigmoid via exp, and an [8,8] group-aggregation
matmul. Final divide once at kernel end.
"""

import os
import sys
import types

import numpy as np

# ---------------------------------------------------------------- constants
B, L = 8192, 16
H, DK, DV = 8, 32, 32
DM, BN, V = 256, 128, 50000
NCORES = 8
P = 128
BC = B // NCORES                  # batches per core (1024)
TOKC = BC * L                     # tokens per core (16384)
NSUB_FULL = TOKC // P             # subtiles per core (128)
GRP = P // L                      # groups per subtile (8)
ST = 4                            # subtiles per block
W = ST * P                        # tokens per block (512)
SCL = 1.0 / np.sqrt(float(DK))
EPS = 1e-5


def _install_ntff_hook():
    """Register the axon NTFF profiling hook if the image's antenv lacks it,
    so run_bass_kernel_spmd(trace=True) works in this container."""
    try:
        import antenv.axon_hooks  # noqa: F401
        return
    except ImportError:
        pass
    try:
        from trn_agent_boot.trn_boot import _ntff_profile_via_ctypes
        hook = _ntff_profile_via_ctypes("/opt/axon/libaxon_pjrt.so")
    except Exception:
        hook = None
    m = types.ModuleType("antenv.axon_hooks")
    m.get_axon_ntff_profile_hook = lambda: hook
    m.set_axon_ntff_profile_hook = lambda h: None
    sys.modules["antenv.axon_hooks"] = m


def _bf16(a):
    import ml_dtypes
    return np.ascontiguousarray(a.astype(ml_dtypes.bfloat16))


def _triv(g, b):
    return bool(np.allclose(g, 1.0, atol=1e-12) and np.allclose(b, 0.0, atol=1e-12))


# ------------------------------------------------------------- host weights
def _prep_consts(w):
    """Fold LN affines into projection weights; build device const arrays."""
    c = {}
    f32 = np.float32

    wq_eff = (np.asarray(w["Wq"], f32) * np.asarray(w["ln1_g"], f32)[None, :]) * SCL
    wk_eff = np.asarray(w["Wk"], f32) * np.asarray(w["ln2_g"], f32)[None, :]
    wv_eff = np.asarray(w["Wv"], f32) * np.asarray(w["ln3_g"], f32)[None, :]
    cq = (np.asarray(w["ln1_b"], f32) @ np.asarray(w["Wq"], f32).T) * SCL
    ck = np.asarray(w["ln2_b"], f32) @ np.asarray(w["Wk"], f32).T
    cv = np.asarray(w["ln3_b"], f32) @ np.asarray(w["Wv"], f32).T

    # this kernel bakes in: zero qkv biases, zero MLP biases, trivial
    # p1/classifier LN affines (all true for the reference init)
    assert np.allclose(cq, 0.0) and np.allclose(ck, 0.0) and np.allclose(cv, 0.0)
    for k in ("p1_b1", "p1_b2", "p2_b1", "p2_b2"):
        assert np.allclose(w[k], 0.0)
    assert _triv(w["p1_lng"], w["p1_lnb"])
    assert _triv(w["lnc1_g"], w["lnc1_b"])
    assert _triv(w["lnc2_g"], w["lnc2_b"])

    # head-masked q weights: wq4[bn, b2*512 + hh*128 + f] =
    #   Wq_eff.T[bn, b2*128+f] if f//32 == hh else 0
    wqT = wq_eff.T                                           # [BN, 256]
    wq4 = np.zeros((BN, 1024), f32)
    for b2 in range(2):
        for hh in range(4):
            sl = slice(hh * 32, hh * 32 + 32)
            wq4[:, b2 * 512 + hh * 128 + hh * 32:
                b2 * 512 + hh * 128 + hh * 32 + 32] = wqT[:, b2 * 128:
                                                          (b2 + 1) * 128][:, sl]
    c["wq4"] = _bf16(wq4)
    c["wk"] = _bf16(wk_eff.T)                                # [BN, 256]

    wv_aug = np.zeros((BN, H * (DV + 1)), f32)               # [128,264]
    for h in range(H):
        wv_aug[:, h * 33:h * 33 + 32] = wv_eff.T[:, h * 32:(h + 1) * 32]
    c["wv"] = _bf16(wv_aug)

    # feature-major (lhsT) weights: lhsT[k, m] with k = input chunk rows
    c["wfc1f"] = _bf16(np.asarray(w["Wfc1"], f32).T)         # [HDV, DM]
    c["p1w1f"] = _bf16(np.asarray(w["p1_w1"], f32).T)        # [DM(in), DM1]
    c["p1w2f"] = _bf16(np.asarray(w["p1_w2"], f32).T)        # [DM1, DM]
    c["p2w1f"] = _bf16(np.asarray(w["p2_w1"], f32).T)        # [BN, DM1]
    c["p2w2f"] = _bf16(np.asarray(w["p2_w2"], f32).T)        # [DM1, DM]

    # merged gather table: [V, 256] = [LN-normalized | raw with row0 zeroed]
    tab = np.asarray(w["node_emb"], f32)
    m = tab.mean(axis=1, keepdims=True)
    v = ((tab - m) ** 2).mean(axis=1, keepdims=True)
    tabn = (tab - m) / np.sqrt(v + EPS)
    tabe = tab.copy()
    tabe[0, :] = 0.0
    c["tabs"] = _bf16(np.concatenate([tabn, tabe], axis=1))  # [V, 256]

    # block-diag(16)-minus-eye multiplicative mask, tiled 8x (8 head slots)
    blk = np.zeros((P, P), f32)
    for g in range(GRP):
        blk[g * L:(g + 1) * L, g * L:(g + 1) * L] = 1.0
    blk -= np.eye(P, dtype=f32)
    blk = np.maximum(blk, 0.0)
    c["mask8"] = _bf16(np.tile(blk, (1, 8)))                 # [128,1024]

    gind = np.zeros((P, GRP), f32)
    for g in range(GRP):
        gind[g * L:(g + 1) * L, g] = 1.0
    c["gind"] = gind

    # stats lhsT blocks: wstat[:, h*50 + q*10 + j], block (q,h) is [128,10]
    # with col 2q = wcls half h, col 2q+1 = ones (others zero); quantity
    # order q: u, st, uu, ss, ust
    wcls = np.asarray(w["Wcls"], f32).reshape(DM)
    wstat = np.zeros((P, 100), f32)
    for h in range(2):
        for q in range(5):
            wstat[:, h * 50 + q * 10 + 2 * q] = wcls[h * P:(h + 1) * P]
            wstat[:, h * 50 + q * 10 + 2 * q + 1] = 1.0
    c["wstat"] = _bf16(wstat)
    c["_swcls"] = float(wcls.sum())

    c["ident"] = _bf16(np.eye(P, dtype=f32))
    c["identf"] = np.eye(16, dtype=f32)

    c["_bcls"] = float(np.asarray(w["bcls"]).reshape(-1)[0])
    return c


# ------------------------------------------------------------ device program
def build_nc(bcls, swcls, n_sub):
    import contextlib

    import concourse.bacc as bacc
    import concourse.tile as tile
    import concourse.mybir as mybir
    from concourse import bass

    dt = mybir.dt
    AF = mybir.ActivationFunctionType
    OP = mybir.AluOpType
    IOA = bass.IndirectOffsetOnAxis
    TB = 4                       # blocks per batched tail
    assert n_sub % (ST * TB) == 0

    nc = bacc.Bacc()

    # ---- dram tensors
    idxc = nc.dram_tensor("idxc", [P, n_sub], dt.int32, kind="ExternalInput")
    npmc = nc.dram_tensor("npmc", [P, n_sub], dt.float32, kind="ExternalInput")
    tabs_d = nc.dram_tensor("tabs", [V, 2 * BN], dt.bfloat16, kind="ExternalInput")
    wq4_d = nc.dram_tensor("wq4", [BN, 1024], dt.bfloat16, kind="ExternalInput")
    wk_d = nc.dram_tensor("wk", [BN, 256], dt.bfloat16, kind="ExternalInput")
    wv_d = nc.dram_tensor("wv", [BN, 264], dt.bfloat16, kind="ExternalInput")
    wfc1_d = nc.dram_tensor("wfc1f", [2 * P, DM], dt.bfloat16, kind="ExternalInput")
    p1w1_d = nc.dram_tensor("p1w1f", [2 * P, DM], dt.bfloat16, kind="ExternalInput")
    p1w2_d = nc.dram_tensor("p1w2f", [2 * P, DM], dt.bfloat16, kind="ExternalInput")
    p2w1_d = nc.dram_tensor("p2w1f", [BN, DM], dt.bfloat16, kind="ExternalInput")
    p2w2_d = nc.dram_tensor("p2w2f", [2 * P, DM], dt.bfloat16, kind="ExternalInput")
    mask_d = nc.dram_tensor("mask8", [P, 1024], dt.bfloat16, kind="ExternalInput")
    gind_d = nc.dram_tensor("gind", [P, GRP], dt.float32, kind="ExternalInput")
    wstat_d = nc.dram_tensor("wstat", [P, 100], dt.bfloat16, kind="ExternalInput")
    ident_d = nc.dram_tensor("ident", [P, P], dt.bfloat16, kind="ExternalInput")
    identf_d = nc.dram_tensor("identf", [16, 16], dt.float32, kind="ExternalInput")
    outp = nc.dram_tensor("outp", [GRP, n_sub], dt.float32, kind="ExternalOutput")

    with tile.TileContext(nc) as tc:
        with contextlib.ExitStack() as ctx:
            singles = ctx.enter_context(tc.tile_pool(name="singles", bufs=1))
            io = ctx.enter_context(tc.tile_pool(name="io", bufs=8))
            sub = ctx.enter_context(tc.tile_pool(name="sub", bufs=4))
            blk = ctx.enter_context(tc.tile_pool(name="blk", bufs=3))
            # PSUM: 8 banks -> big:3 + med:3 + wide:2
            psA = ctx.enter_context(tc.tile_pool(name="psA", bufs=4, space="PSUM"))
            psM = ctx.enter_context(tc.tile_pool(name="psM", bufs=2, space="PSUM"))
            psW = ctx.enter_context(tc.tile_pool(name="psW", bufs=2, space="PSUM"))

            def load(d, shape, dtp):
                t = singles.tile(shape, dtp, name=d.name + "_sb")
                nc.sync.dma_start(t[:], d[:, :])
                return t

            idx_sb = load(idxc, [P, n_sub], dt.int32)
            npm_sb = load(npmc, [P, n_sub], dt.float32)
            wq4 = load(wq4_d, [BN, 1024], dt.bfloat16)
            wk = load(wk_d, [BN, 256], dt.bfloat16)
            wv = load(wv_d, [BN, 264], dt.bfloat16)
            mask_sb = load(mask_d, [P, 1024], dt.bfloat16)
            gind_sb = load(gind_d, [P, GRP], dt.float32)
            wstat_sb = load(wstat_d, [P, 100], dt.bfloat16)
            ident = load(ident_d, [P, P], dt.bfloat16)
            identf = load(identf_d, [16, 16], dt.float32)
            wfc1, p1w1, p1w2, p2w2 = ([None, None] for _ in range(4))
            for k in range(2):
                for nm, arr, d in (("wfc1f", wfc1, wfc1_d), ("p1w1f", p1w1, p1w1_d),
                                   ("p1w2f", p1w2, p1w2_d), ("p2w2f", p2w2, p2w2_d)):
                    arr[k] = singles.tile([P, DM], dt.bfloat16, name=f"{nm}_{k}")
                    nc.sync.dma_start(arr[k][:], d[k * P:(k + 1) * P, :])
            p2w1 = load(p2w1_d, [BN, DM], dt.bfloat16)

            epst = singles.tile([P, 1], dt.float32, name="epst")
            nc.vector.memset(epst[:], EPS)
            res = singles.tile([GRP, 2 * n_sub], dt.float32, name="res")

            def gather(t):
                ne = io.tile([P, 2 * BN], dt.bfloat16, tag="ne", name="ne")
                nc.gpsimd.indirect_dma_start(
                    out=ne[:], out_offset=None, in_=tabs_d[:, :],
                    in_offset=IOA(ap=idx_sb[:, t:t + 1], axis=0))
                return ne

            nblk = n_sub // ST
            ne_cur = [gather(s) for s in range(ST)]
            stT4 = None

            for bb in range(nblk):
                # neT_w: cols 0:512 = nT (s-major), 512:1024 = emT
                neT_w = blk.tile([P, 2 * W], dt.bfloat16, tag="neTw", name="neT_w")
                nT_w, emT_w = neT_w[:, 0:W], neT_w[:, W:2 * W]
                ctxT_w = blk.tile([P, 2 * W], dt.bfloat16, tag="ctxTw",
                                  name="ctxT_w")

                for s in range(ST):
                    # ---- transposes of n and em into the block-wide tile
                    ne = ne_cur[s]
                    ne_ps = psM.tile([P, 2 * P], dt.bfloat16, tag="med", name="ne_ps")
                    nc.tensor.transpose(ne_ps[:, 0:P], ne[:, 0:P], ident[:])
                    nc.tensor.transpose(ne_ps[:, P:2 * P], ne[:, P:2 * P], ident[:])
                    nc.vector.tensor_copy(
                        neT_w[:].rearrange("p (k c) -> p k c", k=2)
                        [:, :, s * P:(s + 1) * P],
                        ne_ps[:].rearrange("p (k c) -> p k c", k=2))

                # prefetch next block's gathers (ahead of gpsimd tail work)
                if bb + 1 < nblk:
                    ne_cur = [gather((bb + 1) * ST + s) for s in range(ST)]

                # ---- block-wide k and head-masked q projections
                # kT_w cols: j*512 + s*128 + t   (j = feature half)
                kT_w = blk.tile([P, 2 * W], dt.bfloat16, tag="kTw", name="kT_w")
                for j in range(2):
                    k_ps = psA.tile([P, W], dt.float32, tag="big", name="k_ps")
                    nc.tensor.matmul(k_ps[:], lhsT=wk[:, j * P:(j + 1) * P],
                                     rhs=nT_w[:])
                    nc.scalar.activation(kT_w[:, j * W:(j + 1) * W], k_ps[:],
                                         AF.Copy)
                # qT4_w[b2] cols: s*512 + hh*128 + t
                qT4_w = [None, None]
                for b2 in range(2):
                    qw = blk.tile([P, 4 * W], dt.bfloat16, tag=f"qw{b2}",
                                  name=f"qw{b2}")
                    for hh in range(4):
                        q_ps = psA.tile([P, W], dt.float32, tag="big", name="q_ps")
                        nc.tensor.matmul(
                            q_ps[:],
                            lhsT=wq4[:, b2 * 512 + hh * P:b2 * 512 + (hh + 1) * P],
                            rhs=nT_w[:])
                        if hh == 3:
                            nc.scalar.activation(
                                qw[:].rearrange("p (s q) -> p s q", q=512)
                                [:, :, hh * P:(hh + 1) * P],
                                q_ps[:].rearrange("p (s t) -> p s t", t=P), AF.Copy)
                        else:
                            nc.vector.tensor_copy(
                                qw[:].rearrange("p (s q) -> p s q", q=512)
                                [:, :, hh * P:(hh + 1) * P],
                                q_ps[:].rearrange("p (s t) -> p s t", t=P))
                    qT4_w[b2] = qw

                for s in range(ST):
                    # ---- v (+ones aug)
                    v_ps = psM.tile([P, 264], dt.float32, tag="med", name="v_ps")
                    nc.tensor.matmul(v_ps[:], lhsT=nT_w[:, s * P:(s + 1) * P],
                                     rhs=wv[:])
                    v_aug = sub.tile([P, 264], dt.bfloat16, tag="v_aug", name="v_aug")
                    nc.scalar.activation(v_aug[:], v_ps[:], AF.Copy)
                    va3 = v_aug[:].rearrange("p (h c) -> p h c", c=33)
                    nc.gpsimd.memset(va3[:, :, 32:33], 1.0)

                    # ---- attention: S^T (4 heads per matmul), exp, mask
                    pt = sub.tile([P, 1024], dt.bfloat16, tag="pt", name="pt")
                    for b2 in range(2):
                        s_ps = psA.tile([P, 512], dt.float32, tag="big", name="s_ps")
                        nc.tensor.matmul(
                            s_ps[:],
                            lhsT=kT_w[:, b2 * W + s * P:b2 * W + (s + 1) * P],
                            rhs=qT4_w[b2][:, s * 512:(s + 1) * 512])
                        nc.scalar.activation(pt[:, b2 * 512:(b2 + 1) * 512], s_ps[:],
                                             AF.Exp)
                    ptm = sub.tile([P, 1024], dt.bfloat16, tag="ptm", name="ptm")
                    nc.vector.tensor_tensor(out=ptm[:, 0:512], in0=pt[:, 0:512],
                                            in1=mask_sb[:, 0:512], op=OP.mult)
                    nc.gpsimd.tensor_tensor(out=ptm[:, 512:1024], in0=pt[:, 512:1024],
                                            in1=mask_sb[:, 512:1024], op=OP.mult)

                    # ---- PV: [ctx~|den] per head, normalize via recip(den)
                    ca_ps = psM.tile([P, 264], dt.float32, tag="med", name="ca_ps")
                    for h in range(H):
                        nc.tensor.matmul(
                            ca_ps[:, h * 33:(h + 1) * 33],
                            lhsT=ptm[:, h * P:(h + 1) * P],
                            rhs=v_aug[:, h * 33:(h + 1) * 33])
                    ca3 = ca_ps[:].rearrange("p (h c) -> p h c", c=33)
                    rec = sub.tile([P, H], dt.float32, tag="rec", name="rec")
                    rec3 = rec[:].rearrange("p (h o) -> p h o", o=1)
                    nc.vector.reciprocal(rec3[:], ca3[:, :, 32:33])
                    ctx_bf = sub.tile([P, 256], dt.bfloat16, tag="ctx", name="ctx_bf")
                    cb3 = ctx_bf[:].rearrange("p (h c) -> p h c", c=32)
                    nc.vector.tensor_tensor(out=cb3[:], in0=ca3[:, :, 0:32],
                                            in1=rec3.to_broadcast([P, H, 32]),
                                            op=OP.mult)

                    # ---- ctx transposed into the block-wide feature-major tile
                    ct_ps = psM.tile([P, 2 * P], dt.bfloat16, tag="med", name="ct_ps")
                    nc.tensor.transpose(ct_ps[:, 0:P], ctx_bf[:, 0:P], ident[:])
                    nc.tensor.transpose(ct_ps[:, P:2 * P], ctx_bf[:, P:2 * P],
                                        ident[:])
                    nc.vector.tensor_copy(
                        ctxT_w[:].rearrange("p (k c) -> p k c", k=2)
                        [:, :, s * P:(s + 1) * P],
                        ct_ps[:].rearrange("p (k c) -> p k c", k=2))

                # ======== block-wide feature-major MLP + stats head ========
                # fc1: dinT[dm_half, tok] (K-accum over 2 hdv chunks)
                dinT = blk.tile([P, 2 * W], dt.bfloat16, tag="dinT", name="dinT")
                din_ps = [None, None]
                for hf in range(2):
                    sc = psW.tile([P, W], dt.float32, tag="wide", name="din_ps")
                    for ch in range(2):
                        nc.tensor.matmul(sc[:], lhsT=wfc1[ch][:, hf * P:(hf + 1) * P],
                                         rhs=ctxT_w[:, ch * W:(ch + 1) * W],
                                         start=(ch == 0), stop=(ch == 1))
                    nc.scalar.activation(dinT[:, hf * W:(hf + 1) * W], sc[:], AF.Copy)
                    din_ps[hf] = sc

                # h1 = tanh(p1w1 @ dinT)
                h1T = blk.tile([P, 2 * W], dt.bfloat16, tag="h1T", name="h1T")
                for hf in range(2):
                    sc = psW.tile([P, W], dt.float32, tag="wide", name="h1_ps")
                    for ch in range(2):
                        nc.tensor.matmul(sc[:], lhsT=p1w1[ch][:, hf * P:(hf + 1) * P],
                                         rhs=dinT[:, ch * W:(ch + 1) * W],
                                         start=(ch == 0), stop=(ch == 1))
                    nc.scalar.activation(h1T[:, hf * W:(hf + 1) * W], sc[:], AF.Tanh)

                # u = dinT + p1w2 @ h1T   (residual re-added from bf16 dinT)
                u_bf = blk.tile([P, 2 * W], dt.bfloat16, tag="u_bf", name="u_bf")
                for hf in range(2):
                    sc = psW.tile([P, W], dt.float32, tag="wide", name="u_ps")
                    for ch in range(2):
                        nc.tensor.matmul(sc[:], lhsT=p1w2[ch][:, hf * P:(hf + 1) * P],
                                         rhs=h1T[:, ch * W:(ch + 1) * W],
                                         start=(ch == 0), stop=(ch == 1))
                    nc.vector.tensor_tensor(out=u_bf[:, hf * W:(hf + 1) * W],
                                            in0=sc[:],
                                            in1=dinT[:, hf * W:(hf + 1) * W],
                                            op=OP.add)

                # static: h2 = tanh(p2w1 @ emT);  st = p2w2 @ h2
                h2T = blk.tile([P, 2 * W], dt.bfloat16, tag="h2T", name="h2T")
                for hf in range(2):
                    sc = psW.tile([P, W], dt.float32, tag="wide", name="h2_ps")
                    nc.tensor.matmul(sc[:], lhsT=p2w1[:, hf * P:(hf + 1) * P],
                                     rhs=emT_w[:])
                    nc.scalar.activation(h2T[:, hf * W:(hf + 1) * W], sc[:], AF.Tanh)
                st_bf = blk.tile([P, 2 * W], dt.bfloat16, tag="st_bf", name="st_bf")
                for hf in range(2):
                    sc = psW.tile([P, W], dt.float32, tag="wide", name="st_ps")
                    for ch in range(2):
                        nc.tensor.matmul(sc[:], lhsT=p2w2[ch][:, hf * P:(hf + 1) * P],
                                         rhs=h2T[:, ch * W:(ch + 1) * W],
                                         start=(ch == 0), stop=(ch == 1))
                    nc.vector.tensor_copy(st_bf[:, hf * W:(hf + 1) * W], sc[:])

                # elementwise products for the second-moment stats
                uu = blk.tile([P, 2 * W], dt.bfloat16, tag="uu", name="uu")
                nc.vector.tensor_tensor(out=uu[:], in0=u_bf[:], in1=u_bf[:],
                                        op=OP.mult)
                ss = blk.tile([P, 2 * W], dt.bfloat16, tag="ss", name="ss")
                nc.vector.tensor_tensor(out=ss[:], in0=st_bf[:], in1=st_bf[:],
                                        op=OP.mult)
                us = blk.tile([P, 2 * W], dt.bfloat16, tag="us", name="us")
                nc.vector.tensor_tensor(out=us[:], in0=u_bf[:], in1=st_bf[:],
                                        op=OP.mult)

                # 10 stats matmuls accumulate into one [10, W] PSUM bank:
                # rows 2q / 2q+1 = wcls-weighted / plain sum of quantity q
                stat_ps = psW.tile([10, W], dt.float32, tag="wide", name="stat_ps")
                qsrc = [u_bf, st_bf, uu, ss, us]
                for i, (q, hf) in enumerate([(q, hf) for q in range(5)
                                             for hf in range(2)]):
                    nc.tensor.matmul(
                        stat_ps[:],
                        lhsT=wstat_sb[:, hf * 50 + q * 10:hf * 50 + q * 10 + 10],
                        rhs=qsrc[q][:, hf * W:(hf + 1) * W],
                        start=(i == 0), stop=(i == 9),
                        skip_group_check=(0 < i < 9))
                stats_sb = blk.tile([10, W], dt.float32, tag="stats", name="stats_sb")
                nc.scalar.activation(stats_sb[:], stat_ps[:], AF.Copy)

                # transpose stats to token-major: statT[p, c*10+q], batched
                # across TB blocks so the scalar tail runs once per TB blocks
                if bb % TB == 0:
                    stT4 = blk.tile([P, TB * 4 * 10], dt.float32, tag="stT",
                                    name="stT4")
                stT_ps = psW.tile([P, 4 * 10], dt.float32, tag="wide", name="stT_ps")
                for cc in range(4):
                    nc.tensor.transpose(stT_ps[:, cc * 10:(cc + 1) * 10],
                                        stats_sb[0:10, cc * P:(cc + 1) * P],
                                        identf[0:10, 0:10])
                nc.vector.tensor_copy(
                    stT4[:, (bb % TB) * 40:(bb % TB) * 40 + 40], stT_ps[:])
                if bb % TB != TB - 1:
                    continue

                NC4 = 4 * TB                    # token chunks in the tail
                sq = stT4[:].rearrange("p (c q) -> p c q", q=10)
                Swu, Su = sq[:, :, 0:1], sq[:, :, 1:2]
                Swst, Sst = sq[:, :, 2:3], sq[:, :, 3:4]
                Swuu, Suu = sq[:, :, 4:5], sq[:, :, 5:6]
                Swss, Sss = sq[:, :, 6:7], sq[:, :, 7:8]
                Swus = sq[:, :, 8:9]

                # means/vars -> rstds  (packed [128, 2*NC4] = (c, k))
                mus = blk.tile([P, 2 * NC4], dt.float32, tag="mus", name="mus")
                mu2 = mus[:].rearrange("p (c k) -> p c k", k=2)
                nc.vector.tensor_scalar_mul(mu2[:, :, 0:1], Su, 1.0 / DM)
                nc.vector.tensor_scalar_mul(mu2[:, :, 1:2], Sst, 1.0 / DM)
                vr = blk.tile([P, 2 * NC4], dt.float32, tag="vr", name="vr")
                vr2 = vr[:].rearrange("p (c k) -> p c k", k=2)
                nc.vector.tensor_scalar_mul(vr2[:, :, 0:1], Suu, 1.0 / DM)
                nc.vector.tensor_scalar_mul(vr2[:, :, 1:2], Sss, 1.0 / DM)
                mm_t = blk.tile([P, 2 * NC4], dt.float32, tag="mm_t", name="mm_t")
                nc.vector.tensor_mul(mm_t[:], mus[:], mus[:])
                nc.vector.tensor_tensor(out=vr[:], in0=vr[:], in1=mm_t[:],
                                        op=OP.subtract)
                std_t = blk.tile([P, 2 * NC4], dt.float32, tag="std_t",
                                 name="std_t")
                nc.scalar.activation(std_t[:], vr[:], AF.Sqrt, bias=epst[:, 0:1])
                rstd = blk.tile([P, 2 * NC4], dt.float32, tag="rstd", name="rstd")
                nc.vector.reciprocal(rstd[:], std_t[:])
                rs2 = rstd[:].rearrange("p (c k) -> p c k", k=2)
                a_v, b_v = rs2[:, :, 0:1], rs2[:, :, 1:2]
                mu_u, mu_s = mu2[:, :, 0:1], mu2[:, :, 1:2]

                # logit = a^2*Swuu + b^2*Swss + Sw*c^2
                #         - 2*(ab*Swus + c*(a*Swu - b*Swst))
                def t4(tag):
                    t = blk.tile([P, NC4], dt.float32, tag=tag, name=tag)
                    return t, t[:].rearrange("p (c o) -> p c o", o=1)

                c_t, c3 = t4("c_t")
                nc.vector.tensor_tensor(out=c3[:], in0=a_v, in1=mu_u, op=OP.mult)
                t1_t, t13 = t4("t1_t")
                nc.vector.tensor_tensor(out=t13[:], in0=b_v, in1=mu_s, op=OP.mult)
                nc.vector.tensor_tensor(out=c3[:], in0=c3[:], in1=t13[:],
                                        op=OP.subtract)
                q_t, q3 = t4("q_t")
                nc.vector.tensor_tensor(out=q3[:], in0=a_v, in1=Swu, op=OP.mult)
                nc.vector.tensor_tensor(out=t13[:], in0=b_v, in1=Swst, op=OP.mult)
                nc.vector.tensor_tensor(out=q3[:], in0=q3[:], in1=t13[:],
                                        op=OP.subtract)
                nc.vector.tensor_tensor(out=q3[:], in0=q3[:], in1=c3[:], op=OP.mult)
                ab_t, ab3 = t4("ab_t")
                nc.vector.tensor_tensor(out=ab3[:], in0=a_v, in1=b_v, op=OP.mult)
                nc.vector.tensor_tensor(out=ab3[:], in0=ab3[:], in1=Swus,
                                        op=OP.mult)
                nc.vector.tensor_tensor(out=q3[:], in0=q3[:], in1=ab3[:], op=OP.add)
                lg_t, lg3 = t4("lg_t")
                nc.vector.tensor_tensor(out=lg3[:], in0=a_v, in1=a_v, op=OP.mult)
                nc.vector.tensor_tensor(out=lg3[:], in0=lg3[:], in1=Swuu,
                                        op=OP.mult)
                nc.vector.tensor_tensor(out=t13[:], in0=b_v, in1=b_v, op=OP.mult)
                nc.vector.tensor_tensor(out=t13[:], in0=t13[:], in1=Swss,
                                        op=OP.mult)
                nc.vector.tensor_tensor(out=lg3[:], in0=lg3[:], in1=t13[:],
                                        op=OP.add)
                nc.vector.tensor_tensor(out=t13[:], in0=c3[:], in1=c3[:], op=OP.mult)
                nc.vector.tensor_scalar_mul(t1_t[:], t1_t[:], float(swcls))
                nc.vector.tensor_tensor(out=lg3[:], in0=lg3[:], in1=t13[:],
                                        op=OP.add)
                nc.vector.tensor_scalar_mul(q_t[:], q_t[:], 2.0)
                nc.vector.tensor_tensor(out=lg3[:], in0=lg3[:], in1=q3[:],
                                        op=OP.subtract)

                e_st = blk.tile([P, NC4], dt.float32, tag="est", name="e_st")
                nc.scalar.activation(e_st[:], lg_t[:], AF.Exp, bias=-bcls,
                                     scale=-1.0)
                pe1 = blk.tile([P, NC4], dt.float32, tag="pe1", name="pe1")
                nc.vector.tensor_scalar_add(pe1[:], e_st[:], 1.0)
                probs_st = blk.tile([P, NC4], dt.float32, tag="pb", name="probs_st")
                nc.vector.reciprocal(probs_st[:], pe1[:])

                npm4 = npm_sb[:, (bb - TB + 1) * ST:(bb + 1) * ST]
                pn_st = blk.tile([P, 2 * NC4], dt.float32, tag="pn", name="pn_st")
                pnv = pn_st[:].rearrange("p (s k) -> p s k", k=2)
                nc.vector.tensor_tensor(
                    out=pnv[:, :, 0:1],
                    in0=probs_st[:].rearrange("p (s o) -> p s o", o=1),
                    in1=npm4.rearrange("p (s o) -> p s o", o=1), op=OP.mult)
                nc.gpsimd.tensor_copy(pnv[:, :, 1:2],
                                      npm4.rearrange("p (s o) -> p s o", o=1))

                agg_ps = psW.tile([GRP, 2 * NC4], dt.float32, tag="wide",
                                  name="agg_ps")
                nc.tensor.matmul(agg_ps[:], lhsT=gind_sb[:], rhs=pn_st[:])
                nc.scalar.activation(
                    res[0:GRP, 2 * NC4 * (bb // TB):2 * NC4 * (bb // TB + 1)],
                    agg_ps[:], AF.Copy)

            # ---- final divide + store
            r3 = res[:].rearrange("p (t k) -> p t k", k=2)
            rn = blk.tile([GRP, n_sub], dt.float32, tag="rn", name="rn")
            rn3 = rn[:].rearrange("p (t o) -> p t o", o=1)
            nc.vector.reciprocal(rn3[:], r3[:, :, 1:2])
            orow = blk.tile([GRP, n_sub], dt.float32, tag="orow", name="orow")
            orow3 = orow[:].rearrange("p (t o) -> p t o", o=1)
            nc.vector.tensor_tensor(out=orow3[:], in0=r3[:, :, 0:1], in1=rn3[:],
                                    op=OP.mult)
            nc.sync.dma_start(outp[:, :], orow[:])

    nc.finalize()
    return nc


# ----------------------------------------------------------------- entry
_NC_CACHE = {}


def kernel(**inputs):
    _install_ntff_hook()
    from concourse.bass_utils import run_bass_kernel_spmd

    n_sub = int(os.environ.get("KBENCH_NSUB", NSUB_FULL))
    consts = _prep_consts(inputs)
    bcls = consts.pop("_bcls")
    swcls = consts.pop("_swcls")

    if n_sub not in _NC_CACHE:
        _NC_CACHE[n_sub] = build_nc(bcls, swcls, n_sub)
    nc = _NC_CACHE[n_sub]

    x = np.asarray(inputs["x"]).astype(np.int32)
    in_maps = []
    for c in range(NCORES):
        xc = x[c * BC:(c + 1) * BC].reshape(-1)          # [16384]
        idxc = np.ascontiguousarray(
            xc[:n_sub * P].reshape(n_sub, P).T)          # [128, n_sub]
        m = {"idxc": idxc, "npmc": (idxc != 0).astype(np.float32)}
        m.update(consts)
        in_maps.append(m)

    trace = bool(int(os.environ.get("KBENCH_TRACE", "0")))
    res = run_bass_kernel_spmd(nc, in_maps, core_ids=list(range(NCORES)),
                               trace=trace)
    kernel._last_results = res

    out = np.zeros((B, 1), np.float32)
    for c in range(NCORES):
        oc = res.results[c]["outp"]                      # [8, n_sub]
        out[c * BC:c * BC + n_sub * GRP, 0] = oc.T.reshape(-1)
    return out


# revision 38
# speedup vs baseline: 1.0730x; 1.0730x over previous
"""Self-contained Trainium2 Bass kernel for nn_Classifier_79929341379065.

kernel(**inputs) takes FULL unsharded inputs (as produced by
reference.setup_inputs()) and returns the FULL [B, 1] float32 output.
Internally: pure data parallel over 8 NeuronCores (batch dim of x),
weights replicated.

Hardcoded shapes: B=8192, L=16, H=8, DK=DV=32, DM=256, BN=128, V=50000.
Per core: 1024 batches = 16384 tokens = 128 subtiles of 128 tokens
(each subtile = 8 attention groups of L=16), processed in blocks of
4 subtiles (512 tokens).

Per subtile (token-major attention, as the tokens arrive from the
gather): one indirect gather from a merged table [V, 256]
(LN-normalized | raw-with-row0-zeroed); PE transposes to feature-major;
k / head-masked q4 projections; S^T (4 heads per matmul, K=128);
exp * blockdiag-minus-eye mask; PV with ones-augmented v giving
[ctx~|den]; normalize; PE-transpose ctx into a block-wide ctxT.

Per block (feature-major, 512-token-wide weight-stationary matmuls):
fc1 -> tanh MLP -> +residual, static tanh MLP from emT, then the
classifier head in closed form: with a=rstd_u, b=rstd_s,
c=a*mu_u-b*mu_s per token,
  logit = a^2*Swuu + b^2*Swss + c^2*Sw - 2*(ab*Swust + c*(a*Swu-b*Swst))
where all S* are partition-contractions computed by 10 matmuls
accumulating into one [10, 512] PSUM bank with [wcls|ones] column
blocks; a 10x128 PE transpose brings the stats token-major for the
final scalar math, sigmoid via exp, and an [8,8] group-aggregation
matmul. Final divide once at kernel end.
"""

import os
import sys
import types

import numpy as np

# ---------------------------------------------------------------- constants
B, L = 8192, 16
H, DK, DV = 8, 32, 32
DM, BN, V = 256, 128, 50000
NCORES = 8
P = 128
BC = B // NCORES                  # batches per core (1024)
TOKC = BC * L                     # tokens per core (16384)
NSUB_FULL = TOKC // P             # subtiles per core (128)
GRP = P // L                      # groups per subtile (8)
ST = 4                            # subtiles per block
W = ST * P                        # tokens per block (512)
SCL = 1.0 / np.sqrt(float(DK))
EPS = 1e-5


def _install_ntff_hook():
    """Register the axon NTFF profiling hook if the image's antenv lacks it,
    so run_bass_kernel_spmd(trace=True) works in this container."""
    try:
        import antenv.axon_hooks  # noqa: F401
        return
    except ImportError:
        pass
    try:
        from trn_agent_boot.trn_boot import _ntff_profile_via_ctypes
        hook = _ntff_profile_via_ctypes("/opt/axon/libaxon_pjrt.so")
    except Exception:
        hook = None
    m = types.ModuleType("antenv.axon_hooks")
    m.get_axon_ntff_profile_hook = lambda: hook
    m.set_axon_ntff_profile_hook = lambda h: None
    sys.modules["antenv.axon_hooks"] = m


def _bf16(a):
    import ml_dtypes
    return np.ascontiguousarray(a.astype(ml_dtypes.bfloat16))


def _triv(g, b):
    return bool(np.allclose(g, 1.0, atol=1e-12) and np.allclose(b, 0.0, atol=1e-12))


# ------------------------------------------------------------- host weights
def _prep_consts(w):
    """Fold LN affines into projection weights; build device const arrays."""
    c = {}
    f32 = np.float32

    wq_eff = (np.asarray(w["Wq"], f32) * np.asarray(w["ln1_g"], f32)[None, :]) * SCL
    wk_eff = np.asarray(w["Wk"], f32) * np.asarray(w["ln2_g"], f32)[None, :]
    wv_eff = np.asarray(w["Wv"], f32) * np.asarray(w["ln3_g"], f32)[None, :]
    cq = (np.asarray(w["ln1_b"], f32) @ np.asarray(w["Wq"], f32).T) * SCL
    ck = np.asarray(w["ln2_b"], f32) @ np.asarray(w["Wk"], f32).T
    cv = np.asarray(w["ln3_b"], f32) @ np.asarray(w["Wv"], f32).T

    # this kernel bakes in: zero qkv biases, zero MLP biases, trivial
    # p1/classifier LN affines (all true for the reference init)
    assert np.allclose(cq, 0.0) and np.allclose(ck, 0.0) and np.allclose(cv, 0.0)
    for k in ("p1_b1", "p1_b2", "p2_b1", "p2_b2"):
        assert np.allclose(w[k], 0.0)
    assert _triv(w["p1_lng"], w["p1_lnb"])
    assert _triv(w["lnc1_g"], w["lnc1_b"])
    assert _triv(w["lnc2_g"], w["lnc2_b"])

    # head-masked q weights: wq4[bn, b2*512 + hh*128 + f] =
    #   Wq_eff.T[bn, b2*128+f] if f//32 == hh else 0
    wqT = wq_eff.T                                           # [BN, 256]
    wq4 = np.zeros((BN, 1024), f32)
    for b2 in range(2):
        for hh in range(4):
            sl = slice(hh * 32, hh * 32 + 32)
            wq4[:, b2 * 512 + hh * 128 + hh * 32:
                b2 * 512 + hh * 128 + hh * 32 + 32] = wqT[:, b2 * 128:
                                                          (b2 + 1) * 128][:, sl]
    c["wq4"] = _bf16(wq4)
    c["wk"] = _bf16(wk_eff.T)                                # [BN, 256]

    wv_aug = np.zeros((BN, H * (DV + 1)), f32)               # [128,264]
    for h in range(H):
        wv_aug[:, h * 33:h * 33 + 32] = wv_eff.T[:, h * 32:(h + 1) * 32]
    c["wv"] = _bf16(wv_aug)

    # feature-major (lhsT) weights: lhsT[k, m] with k = input chunk rows
    c["wfc1f"] = _bf16(np.asarray(w["Wfc1"], f32).T)         # [HDV, DM]
    c["p1w1f"] = _bf16(np.asarray(w["p1_w1"], f32).T)        # [DM(in), DM1]
    c["p1w2f"] = _bf16(np.asarray(w["p1_w2"], f32).T)        # [DM1, DM]
    c["p2w1f"] = _bf16(np.asarray(w["p2_w1"], f32).T)        # [BN, DM1]
    c["p2w2f"] = _bf16(np.asarray(w["p2_w2"], f32).T)        # [DM1, DM]

    # merged gather table: [V, 256] = [LN-normalized | raw with row0 zeroed]
    tab = np.asarray(w["node_emb"], f32)
    m = tab.mean(axis=1, keepdims=True)
    v = ((tab - m) ** 2).mean(axis=1, keepdims=True)
    tabn = (tab - m) / np.sqrt(v + EPS)
    tabe = tab.copy()
    tabe[0, :] = 0.0
    c["tabs"] = _bf16(np.concatenate([tabn, tabe], axis=1))  # [V, 256]

    # block-diag(16)-minus-eye multiplicative mask, tiled 8x (8 head slots)
    blk = np.zeros((P, P), f32)
    for g in range(GRP):
        blk[g * L:(g + 1) * L, g * L:(g + 1) * L] = 1.0
    blk -= np.eye(P, dtype=f32)
    blk = np.maximum(blk, 0.0)
    c["mask8"] = _bf16(np.tile(blk, (1, 8)))                 # [128,1024]

    gind = np.zeros((P, GRP), f32)
    for g in range(GRP):
        gind[g * L:(g + 1) * L, g] = 1.0
    c["gind"] = gind

    # stats lhsT blocks: wstat[:, h*50 + q*10 + j], block (q,h) is [128,10]
    # with col 2q = wcls half h, col 2q+1 = ones (others zero); quantity
    # order q: u, st, uu, ss, ust
    wcls = np.asarray(w["Wcls"], f32).reshape(DM)
    wstat = np.zeros((P, 100), f32)
    for h in range(2):
        for q in range(5):
            wstat[:, h * 50 + q * 10 + 2 * q] = wcls[h * P:(h + 1) * P]
            wstat[:, h * 50 + q * 10 + 2 * q + 1] = 1.0
    c["wstat"] = _bf16(wstat)
    c["_swcls"] = float(wcls.sum())

    c["ident"] = _bf16(np.eye(P, dtype=f32))
    c["identf"] = np.eye(16, dtype=f32)

    c["_bcls"] = float(np.asarray(w["bcls"]).reshape(-1)[0])
    return c


# ------------------------------------------------------------ device program
def build_nc(bcls, swcls, n_sub):
    import contextlib

    import concourse.bacc as bacc
    import concourse.tile as tile
    import concourse.mybir as mybir
    from concourse import bass

    dt = mybir.dt
    AF = mybir.ActivationFunctionType
    OP = mybir.AluOpType
    IOA = bass.IndirectOffsetOnAxis
    TB = 4                       # blocks per batched tail
    assert n_sub % (ST * TB) == 0

    nc = bacc.Bacc()

    # ---- dram tensors
    idxc = nc.dram_tensor("idxc", [P, n_sub], dt.int32, kind="ExternalInput")
    npmc = nc.dram_tensor("npmc", [P, n_sub], dt.float32, kind="ExternalInput")
    tabs_d = nc.dram_tensor("tabs", [V, 2 * BN], dt.bfloat16, kind="ExternalInput")
    wq4_d = nc.dram_tensor("wq4", [BN, 1024], dt.bfloat16, kind="ExternalInput")
    wk_d = nc.dram_tensor("wk", [BN, 256], dt.bfloat16, kind="ExternalInput")
    wv_d = nc.dram_tensor("wv", [BN, 264], dt.bfloat16, kind="ExternalInput")
    wfc1_d = nc.dram_tensor("wfc1f", [2 * P, DM], dt.bfloat16, kind="ExternalInput")
    p1w1_d = nc.dram_tensor("p1w1f", [2 * P, DM], dt.bfloat16, kind="ExternalInput")
    p1w2_d = nc.dram_tensor("p1w2f", [2 * P, DM], dt.bfloat16, kind="ExternalInput")
    p2w1_d = nc.dram_tensor("p2w1f", [BN, DM], dt.bfloat16, kind="ExternalInput")
    p2w2_d = nc.dram_tensor("p2w2f", [2 * P, DM], dt.bfloat16, kind="ExternalInput")
    mask_d = nc.dram_tensor("mask8", [P, 1024], dt.bfloat16, kind="ExternalInput")
    gind_d = nc.dram_tensor("gind", [P, GRP], dt.float32, kind="ExternalInput")
    wstat_d = nc.dram_tensor("wstat", [P, 100], dt.bfloat16, kind="ExternalInput")
    ident_d = nc.dram_tensor("ident", [P, P], dt.bfloat16, kind="ExternalInput")
    identf_d = nc.dram_tensor("identf", [16, 16], dt.float32, kind="ExternalInput")
    outp = nc.dram_tensor("outp", [GRP, n_sub], dt.float32, kind="ExternalOutput")

    with tile.TileContext(nc) as tc:
        with contextlib.ExitStack() as ctx:
            singles = ctx.enter_context(tc.tile_pool(name="singles", bufs=1))
            io = ctx.enter_context(tc.tile_pool(name="io", bufs=10))
            sub = ctx.enter_context(tc.tile_pool(name="sub", bufs=5))
            blk = ctx.enter_context(tc.tile_pool(name="blk", bufs=3))
            # PSUM: 8 banks -> big:3 + med:3 + wide:2
            psA = ctx.enter_context(tc.tile_pool(name="psA", bufs=3, space="PSUM"))
            psM = ctx.enter_context(tc.tile_pool(name="psM", bufs=3, space="PSUM"))
            psW = ctx.enter_context(tc.tile_pool(name="psW", bufs=2, space="PSUM"))

            def load(d, shape, dtp):
                t = singles.tile(shape, dtp, name=d.name + "_sb")
                nc.sync.dma_start(t[:], d[:, :])
                return t

            idx_sb = load(idxc, [P, n_sub], dt.int32)
            npm_sb = load(npmc, [P, n_sub], dt.float32)
            wq4 = load(wq4_d, [BN, 1024], dt.bfloat16)
            wk = load(wk_d, [BN, 256], dt.bfloat16)
            wv = load(wv_d, [BN, 264], dt.bfloat16)
            mask_sb = load(mask_d, [P, 1024], dt.bfloat16)
            gind_sb = load(gind_d, [P, GRP], dt.float32)
            wstat_sb = load(wstat_d, [P, 100], dt.bfloat16)
            ident = load(ident_d, [P, P], dt.bfloat16)
            identf = load(identf_d, [16, 16], dt.float32)
            wfc1, p1w1, p1w2, p2w2 = ([None, None] for _ in range(4))
            for k in range(2):
                for nm, arr, d in (("wfc1f", wfc1, wfc1_d), ("p1w1f", p1w1, p1w1_d),
                                   ("p1w2f", p1w2, p1w2_d), ("p2w2f", p2w2, p2w2_d)):
                    arr[k] = singles.tile([P, DM], dt.bfloat16, name=f"{nm}_{k}")
                    nc.sync.dma_start(arr[k][:], d[k * P:(k + 1) * P, :])
            p2w1 = load(p2w1_d, [BN, DM], dt.bfloat16)

            epst = singles.tile([P, 1], dt.float32, name="epst")
            nc.vector.memset(epst[:], EPS)
            res = singles.tile([GRP, 2 * n_sub], dt.float32, name="res")

            def gather(t):
                ne = io.tile([P, 2 * BN], dt.bfloat16, tag="ne", name="ne")
                nc.gpsimd.indirect_dma_start(
                    out=ne[:], out_offset=None, in_=tabs_d[:, :],
                    in_offset=IOA(ap=idx_sb[:, t:t + 1], axis=0))
                return ne

            nblk = n_sub // ST
            ne_cur = [gather(s) for s in range(ST)]
            stT4 = None

            for bb in range(nblk):
                # neT_w: cols 0:512 = nT (s-major), 512:1024 = emT
                neT_w = blk.tile([P, 2 * W], dt.bfloat16, tag="neTw", name="neT_w")
                nT_w, emT_w = neT_w[:, 0:W], neT_w[:, W:2 * W]
                ctxT_w = blk.tile([P, 2 * W], dt.bfloat16, tag="ctxTw",
                                  name="ctxT_w")

                for s in range(ST):
                    # ---- transposes of n and em into the block-wide tile
                    ne = ne_cur[s]
                    ne_ps = psM.tile([P, 2 * P], dt.bfloat16, tag="med", name="ne_ps")
                    nc.tensor.transpose(ne_ps[:, 0:P], ne[:, 0:P], ident[:])
                    nc.tensor.transpose(ne_ps[:, P:2 * P], ne[:, P:2 * P], ident[:])
                    nc.vector.tensor_copy(
                        neT_w[:].rearrange("p (k c) -> p k c", k=2)
                        [:, :, s * P:(s + 1) * P],
                        ne_ps[:].rearrange("p (k c) -> p k c", k=2))

                # prefetch next block's gathers (ahead of gpsimd tail work)
                if bb + 1 < nblk:
                    ne_cur = [gather((bb + 1) * ST + s) for s in range(ST)]

                # ---- block-wide k and head-masked q projections
                # kT_w cols: j*512 + s*128 + t   (j = feature half)
                kT_w = blk.tile([P, 2 * W], dt.bfloat16, tag="kTw", name="kT_w")
                for j in range(2):
                    k_ps = psA.tile([P, W], dt.float32, tag="big", name="k_ps")
                    nc.tensor.matmul(k_ps[:], lhsT=wk[:, j * P:(j + 1) * P],
                                     rhs=nT_w[:])
                    nc.scalar.activation(kT_w[:, j * W:(j + 1) * W], k_ps[:],
                                         AF.Copy)
                # qT4_w[b2] cols: s*512 + hh*128 + t
                qT4_w = [None, None]
                for b2 in range(2):
                    qw = blk.tile([P, 4 * W], dt.bfloat16, tag=f"qw{b2}",
                                  name=f"qw{b2}")
                    for hh in range(4):
                        q_ps = psA.tile([P, W], dt.float32, tag="big", name="q_ps")
                        nc.tensor.matmul(
                            q_ps[:],
                            lhsT=wq4[:, b2 * 512 + hh * P:b2 * 512 + (hh + 1) * P],
                            rhs=nT_w[:])
                        if hh == 3:
                            nc.scalar.activation(
                                qw[:].rearrange("p (s q) -> p s q", q=512)
                                [:, :, hh * P:(hh + 1) * P],
                                q_ps[:].rearrange("p (s t) -> p s t", t=P), AF.Copy)
                        else:
                            nc.vector.tensor_copy(
                                qw[:].rearrange("p (s q) -> p s q", q=512)
                                [:, :, hh * P:(hh + 1) * P],
                                q_ps[:].rearrange("p (s t) -> p s t", t=P))
                    qT4_w[b2] = qw

                for s in range(ST):
                    # ---- v (+ones aug)
                    v_ps = psM.tile([P, 264], dt.float32, tag="med", name="v_ps")
                    nc.tensor.matmul(v_ps[:], lhsT=nT_w[:, s * P:(s + 1) * P],
                                     rhs=wv[:])
                    v_aug = sub.tile([P, 264], dt.bfloat16, tag="v_aug", name="v_aug")
                    if s % 2 == 0:
                        nc.scalar.activation(v_aug[:], v_ps[:], AF.Copy)
                    else:
                        nc.vector.tensor_copy(v_aug[:], v_ps[:])
                    va3 = v_aug[:].rearrange("p (h c) -> p h c", c=33)
                    nc.gpsimd.memset(va3[:, :, 32:33], 1.0)

                    # ---- attention: S^T (4 heads per matmul), exp, mask
                    pt = sub.tile([P, 1024], dt.bfloat16, tag="pt", name="pt")
                    for b2 in range(2):
                        s_ps = psA.tile([P, 512], dt.float32, tag="big", name="s_ps")
                        nc.tensor.matmul(
                            s_ps[:],
                            lhsT=kT_w[:, b2 * W + s * P:b2 * W + (s + 1) * P],
                            rhs=qT4_w[b2][:, s * 512:(s + 1) * 512])
                        nc.scalar.activation(pt[:, b2 * 512:(b2 + 1) * 512], s_ps[:],
                                             AF.Exp)
                    ptm = sub.tile([P, 1024], dt.bfloat16, tag="ptm", name="ptm")
                    nc.vector.tensor_tensor(out=ptm[:, 0:512], in0=pt[:, 0:512],
                                            in1=mask_sb[:, 0:512], op=OP.mult)
                    nc.gpsimd.tensor_tensor(out=ptm[:, 512:1024], in0=pt[:, 512:1024],
                                            in1=mask_sb[:, 512:1024], op=OP.mult)

                    # ---- PV: [ctx~|den] per head, normalize via recip(den)
                    ca_ps = psM.tile([P, 264], dt.float32, tag="med", name="ca_ps")
                    for h in range(H):
                        nc.tensor.matmul(
                            ca_ps[:, h * 33:(h + 1) * 33],
                            lhsT=ptm[:, h * P:(h + 1) * P],
                            rhs=v_aug[:, h * 33:(h + 1) * 33])
                    ca3 = ca_ps[:].rearrange("p (h c) -> p h c", c=33)
                    rec = sub.tile([P, H], dt.float32, tag="rec", name="rec")
                    rec3 = rec[:].rearrange("p (h o) -> p h o", o=1)
                    nc.vector.reciprocal(rec3[:], ca3[:, :, 32:33])
                    ctx_bf = sub.tile([P, 256], dt.bfloat16, tag="ctx", name="ctx_bf")
                    cb3 = ctx_bf[:].rearrange("p (h c) -> p h c", c=32)
                    nc.vector.tensor_tensor(out=cb3[:], in0=ca3[:, :, 0:32],
                                            in1=rec3.to_broadcast([P, H, 32]),
                                            op=OP.mult)

                    # ---- ctx transposed into the block-wide feature-major tile
                    ct_ps = psM.tile([P, 2 * P], dt.bfloat16, tag="med", name="ct_ps")
                    nc.tensor.transpose(ct_ps[:, 0:P], ctx_bf[:, 0:P], ident[:])
                    nc.tensor.transpose(ct_ps[:, P:2 * P], ctx_bf[:, P:2 * P],
                                        ident[:])
                    nc.vector.tensor_copy(
                        ctxT_w[:].rearrange("p (k c) -> p k c", k=2)
                        [:, :, s * P:(s + 1) * P],
                        ct_ps[:].rearrange("p (k c) -> p k c", k=2))

                # ======== block-wide feature-major MLP + stats head ========
                # fc1: dinT[dm_half, tok] (K-accum over 2 hdv chunks)
                dinT = blk.tile([P, 2 * W], dt.bfloat16, tag="dinT", name="dinT")
                din_ps = [None, None]
                for hf in range(2):
                    sc = psW.tile([P, W], dt.float32, tag="wide", name="din_ps")
                    for ch in range(2):
                        nc.tensor.matmul(sc[:], lhsT=wfc1[ch][:, hf * P:(hf + 1) * P],
                                         rhs=ctxT_w[:, ch * W:(ch + 1) * W],
                                         start=(ch == 0), stop=(ch == 1))
                    nc.scalar.activation(dinT[:, hf * W:(hf + 1) * W], sc[:], AF.Copy)
                    din_ps[hf] = sc

                # h1 = tanh(p1w1 @ dinT)
                h1T = blk.tile([P, 2 * W], dt.bfloat16, tag="h1T", name="h1T")
                for hf in range(2):
                    sc = psW.tile([P, W], dt.float32, tag="wide", name="h1_ps")
                    for ch in range(2):
                        nc.tensor.matmul(sc[:], lhsT=p1w1[ch][:, hf * P:(hf + 1) * P],
                                         rhs=dinT[:, ch * W:(ch + 1) * W],
                                         start=(ch == 0), stop=(ch == 1))
                    nc.scalar.activation(h1T[:, hf * W:(hf + 1) * W], sc[:], AF.Tanh)

                # u = dinT + p1w2 @ h1T   (residual re-added from bf16 dinT)
                u_bf = blk.tile([P, 2 * W], dt.bfloat16, tag="u_bf", name="u_bf")
                for hf in range(2):
                    sc = psW.tile([P, W], dt.float32, tag="wide", name="u_ps")
                    for ch in range(2):
                        nc.tensor.matmul(sc[:], lhsT=p1w2[ch][:, hf * P:(hf + 1) * P],
                                         rhs=h1T[:, ch * W:(ch + 1) * W],
                                         start=(ch == 0), stop=(ch == 1))
                    nc.vector.tensor_tensor(out=u_bf[:, hf * W:(hf + 1) * W],
                                            in0=sc[:],
                                            in1=dinT[:, hf * W:(hf + 1) * W],
                                            op=OP.add)

                # static: h2 = tanh(p2w1 @ emT);  st = p2w2 @ h2
                h2T = blk.tile([P, 2 * W], dt.bfloat16, tag="h2T", name="h2T")
                for hf in range(2):
                    sc = psW.tile([P, W], dt.float32, tag="wide", name="h2_ps")
                    nc.tensor.matmul(sc[:], lhsT=p2w1[:, hf * P:(hf + 1) * P],
                                     rhs=emT_w[:])
                    nc.scalar.activation(h2T[:, hf * W:(hf + 1) * W], sc[:], AF.Tanh)
                st_bf = blk.tile([P, 2 * W], dt.bfloat16, tag="st_bf", name="st_bf")
                for hf in range(2):
                    sc = psW.tile([P, W], dt.float32, tag="wide", name="st_ps")
                    for ch in range(2):
                        nc.tensor.matmul(sc[:], lhsT=p2w2[ch][:, hf * P:(hf + 1) * P],
                                         rhs=h2T[:, ch * W:(ch + 1) * W],
                                         start=(ch == 0), stop=(ch == 1))
                    nc.vector.tensor_copy(st_bf[:, hf * W:(hf + 1) * W], sc[:])

                # elementwise products for the second-moment stats
                uu = blk.tile([P, 2 * W], dt.bfloat16, tag="uu", name="uu")
                nc.vector.tensor_tensor(out=uu[:], in0=u_bf[:], in1=u_bf[:],
                                        op=OP.mult)
                ss = blk.tile([P, 2 * W], dt.bfloat16, tag="ss", name="ss")
                nc.vector.tensor_tensor(out=ss[:], in0=st_bf[:], in1=st_bf[:],
                                        op=OP.mult)
                us = blk.tile([P, 2 * W], dt.bfloat16, tag="us", name="us")
                nc.vector.tensor_tensor(out=us[:], in0=u_bf[:], in1=st_bf[:],
                                        op=OP.mult)

                # 10 stats matmuls accumulate into one [10, W] PSUM bank:
                # rows 2q / 2q+1 = wcls-weighted / plain sum of quantity q
                stat_ps = psW.tile([10, W], dt.float32, tag="wide", name="stat_ps")
                qsrc = [u_bf, st_bf, uu, ss, us]
                for i, (q, hf) in enumerate([(q, hf) for q in range(5)
                                             for hf in range(2)]):
                    nc.tensor.matmul(
                        stat_ps[:],
                        lhsT=wstat_sb[:, hf * 50 + q * 10:hf * 50 + q * 10 + 10],
                        rhs=qsrc[q][:, hf * W:(hf + 1) * W],
                        start=(i == 0), stop=(i == 9),
                        skip_group_check=(0 < i < 9))
                stats_sb = blk.tile([10, W], dt.float32, tag="stats", name="stats_sb")
                nc.scalar.activation(stats_sb[:], stat_ps[:], AF.Copy)

                # transpose stats to token-major: statT[p, c*10+q], batched
                # across TB blocks so the scalar tail runs once per TB blocks
                if bb % TB == 0:
                    stT4 = blk.tile([P, TB * 4 * 10], dt.float32, tag="stT",
                                    name="stT4")
                stT_ps = psW.tile([P, 4 * 10], dt.float32, tag="wide", name="stT_ps")
                for cc in range(4):
                    nc.tensor.transpose(stT_ps[:, cc * 10:(cc + 1) * 10],
                                        stats_sb[0:10, cc * P:(cc + 1) * P],
                                        identf[0:10, 0:10])
                nc.vector.tensor_copy(
                    stT4[:, (bb % TB) * 40:(bb % TB) * 40 + 40], stT_ps[:])
                if bb % TB != TB - 1:
                    continue

                NC4 = 4 * TB                    # token chunks in the tail
                sq = stT4[:].rearrange("p (c q) -> p c q", q=10)
                Swu, Su = sq[:, :, 0:1], sq[:, :, 1:2]
                Swst, Sst = sq[:, :, 2:3], sq[:, :, 3:4]
                Swuu, Suu = sq[:, :, 4:5], sq[:, :, 5:6]
                Swss, Sss = sq[:, :, 6:7], sq[:, :, 7:8]
                Swus = sq[:, :, 8:9]

                # means/vars -> rstds  (packed [128, 2*NC4] = (c, k))
                mus = blk.tile([P, 2 * NC4], dt.float32, tag="mus", name="mus")
                mu2 = mus[:].rearrange("p (c k) -> p c k", k=2)
                nc.vector.tensor_scalar_mul(mu2[:, :, 0:1], Su, 1.0 / DM)
                nc.vector.tensor_scalar_mul(mu2[:, :, 1:2], Sst, 1.0 / DM)
                vr = blk.tile([P, 2 * NC4], dt.float32, tag="vr", name="vr")
                vr2 = vr[:].rearrange("p (c k) -> p c k", k=2)
                nc.vector.tensor_scalar_mul(vr2[:, :, 0:1], Suu, 1.0 / DM)
                nc.vector.tensor_scalar_mul(vr2[:, :, 1:2], Sss, 1.0 / DM)
                mm_t = blk.tile([P, 2 * NC4], dt.float32, tag="mm_t", name="mm_t")
                nc.vector.tensor_mul(mm_t[:], mus[:], mus[:])
                nc.vector.tensor_tensor(out=vr[:], in0=vr[:], in1=mm_t[:],
                                        op=OP.subtract)
                std_t = blk.tile([P, 2 * NC4], dt.float32, tag="std_t",
                                 name="std_t")
                nc.scalar.activation(std_t[:], vr[:], AF.Sqrt, bias=epst[:, 0:1])
                rstd = blk.tile([P, 2 * NC4], dt.float32, tag="rstd", name="rstd")
                nc.vector.reciprocal(rstd[:], std_t[:])
                rs2 = rstd[:].rearrange("p (c k) -> p c k", k=2)
                a_v, b_v = rs2[:, :, 0:1], rs2[:, :, 1:2]
                mu_u, mu_s = mu2[:, :, 0:1], mu2[:, :, 1:2]

                # logit = a^2*Swuu + b^2*Swss + Sw*c^2
                #         - 2*(ab*Swus + c*(a*Swu - b*Swst))
                def t4(tag):
                    t = blk.tile([P, NC4], dt.float32, tag=tag, name=tag)
                    return t, t[:].rearrange("p (c o) -> p c o", o=1)

                c_t, c3 = t4("c_t")
                nc.vector.tensor_tensor(out=c3[:], in0=a_v, in1=mu_u, op=OP.mult)
                t1_t, t13 = t4("t1_t")
                nc.vector.tensor_tensor(out=t13[:], in0=b_v, in1=mu_s, op=OP.mult)
                nc.vector.tensor_tensor(out=c3[:], in0=c3[:], in1=t13[:],
                                        op=OP.subtract)
                q_t, q3 = t4("q_t")
                nc.vector.tensor_tensor(out=q3[:], in0=a_v, in1=Swu, op=OP.mult)
                nc.vector.tensor_tensor(out=t13[:], in0=b_v, in1=Swst, op=OP.mult)
                nc.vector.tensor_tensor(out=q3[:], in0=q3[:], in1=t13[:],
                                        op=OP.subtract)
                nc.vector.tensor_tensor(out=q3[:], in0=q3[:], in1=c3[:], op=OP.mult)
                ab_t, ab3 = t4("ab_t")
                nc.vector.tensor_tensor(out=ab3[:], in0=a_v, in1=b_v, op=OP.mult)
                nc.vector.tensor_tensor(out=ab3[:], in0=ab3[:], in1=Swus,
                                        op=OP.mult)
                nc.vector.tensor_tensor(out=q3[:], in0=q3[:], in1=ab3[:], op=OP.add)
                lg_t, lg3 = t4("lg_t")
                nc.vector.tensor_tensor(out=lg3[:], in0=a_v, in1=a_v, op=OP.mult)
                nc.vector.tensor_tensor(out=lg3[:], in0=lg3[:], in1=Swuu,
                                        op=OP.mult)
                nc.vector.tensor_tensor(out=t13[:], in0=b_v, in1=b_v, op=OP.mult)
                nc.vector.tensor_tensor(out=t13[:], in0=t13[:], in1=Swss,
                                        op=OP.mult)
                nc.vector.tensor_tensor(out=lg3[:], in0=lg3[:], in1=t13[:],
                                        op=OP.add)
                nc.vector.tensor_tensor(out=t13[:], in0=c3[:], in1=c3[:], op=OP.mult)
                nc.vector.tensor_scalar_mul(t1_t[:], t1_t[:], float(swcls))
                nc.vector.tensor_tensor(out=lg3[:], in0=lg3[:], in1=t13[:],
                                        op=OP.add)
                nc.vector.tensor_scalar_mul(q_t[:], q_t[:], 2.0)
                nc.vector.tensor_tensor(out=lg3[:], in0=lg3[:], in1=q3[:],
                                        op=OP.subtract)

                e_st = blk.tile([P, NC4], dt.float32, tag="est", name="e_st")
                nc.scalar.activation(e_st[:], lg_t[:], AF.Exp, bias=-bcls,
                                     scale=-1.0)
                pe1 = blk.tile([P, NC4], dt.float32, tag="pe1", name="pe1")
                nc.vector.tensor_scalar_add(pe1[:], e_st[:], 1.0)
                probs_st = blk.tile([P, NC4], dt.float32, tag="pb", name="probs_st")
                nc.vector.reciprocal(probs_st[:], pe1[:])

                npm4 = npm_sb[:, (bb - TB + 1) * ST:(bb + 1) * ST]
                pn_st = blk.tile([P, 2 * NC4], dt.float32, tag="pn", name="pn_st")
                pnv = pn_st[:].rearrange("p (s k) -> p s k", k=2)
                nc.vector.tensor_tensor(
                    out=pnv[:, :, 0:1],
                    in0=probs_st[:].rearrange("p (s o) -> p s o", o=1),
                    in1=npm4.rearrange("p (s o) -> p s o", o=1), op=OP.mult)
                nc.gpsimd.tensor_copy(pnv[:, :, 1:2],
                                      npm4.rearrange("p (s o) -> p s o", o=1))

                agg_ps = psW.tile([GRP, 2 * NC4], dt.float32, tag="wide",
                                  name="agg_ps")
                nc.tensor.matmul(agg_ps[:], lhsT=gind_sb[:], rhs=pn_st[:])
                nc.scalar.activation(
                    res[0:GRP, 2 * NC4 * (bb // TB):2 * NC4 * (bb // TB + 1)],
                    agg_ps[:], AF.Copy)

            # ---- final divide + store
            r3 = res[:].rearrange("p (t k) -> p t k", k=2)
            rn = blk.tile([GRP, n_sub], dt.float32, tag="rn", name="rn")
            rn3 = rn[:].rearrange("p (t o) -> p t o", o=1)
            nc.vector.reciprocal(rn3[:], r3[:, :, 1:2])
            orow = blk.tile([GRP, n_sub], dt.float32, tag="orow", name="orow")
            orow3 = orow[:].rearrange("p (t o) -> p t o", o=1)
            nc.vector.tensor_tensor(out=orow3[:], in0=r3[:, :, 0:1], in1=rn3[:],
                                    op=OP.mult)
            nc.sync.dma_start(outp[:, :], orow[:])

    nc.finalize()
    return nc


# ----------------------------------------------------------------- entry
_NC_CACHE = {}


def kernel(**inputs):
    _install_ntff_hook()
    from concourse.bass_utils import run_bass_kernel_spmd

    n_sub = int(os.environ.get("KBENCH_NSUB", NSUB_FULL))
    consts = _prep_consts(inputs)
    bcls = consts.pop("_bcls")
    swcls = consts.pop("_swcls")

    if n_sub not in _NC_CACHE:
        _NC_CACHE[n_sub] = build_nc(bcls, swcls, n_sub)
    nc = _NC_CACHE[n_sub]

    x = np.asarray(inputs["x"]).astype(np.int32)
    in_maps = []
    for c in range(NCORES):
        xc = x[c * BC:(c + 1) * BC].reshape(-1)          # [16384]
        idxc = np.ascontiguousarray(
            xc[:n_sub * P].reshape(n_sub, P).T)          # [128, n_sub]
        m = {"idxc": idxc, "npmc": (idxc != 0).astype(np.float32)}
        m.update(consts)
        in_maps.append(m)

    trace = bool(int(os.environ.get("KBENCH_TRACE", "0")))
    res = run_bass_kernel_spmd(nc, in_maps, core_ids=list(range(NCORES)),
                               trace=trace)
    kernel._last_results = res

    out = np.zeros((B, 1), np.float32)
    for c in range(NCORES):
        oc = res.results[c]["outp"]                      # [8, n_sub]
        out[c * BC:c * BC + n_sub * GRP, 0] = oc.T.reshape(-1)
    return out
